# revision 5
# baseline (speedup 1.0000x reference)
"""Trainium2 Bass kernel for AngelLoss (center loss + angular loss).

loss = 0.5*sum((feat - centers[y])^2)/B
     + sum_offdiag((c_i.c_j/(|c_i||c_j|) - ct)^2) / (0.5*C*(C-1))

Sharding (8 NeuronCores):
  - batch term: feat/y sharded along batch (8192 rows/core); each core
    dma_gathers its centers rows (bf16 table) and reduces to partials.
  - angular term: center rows sharded (125 rows/core); each core computes
    its 125x1000 slice of the normalized Gram matrix.
  - per-core [1,16] partial sums are combined on the host.
"""

from contextlib import ExitStack

import numpy as np

import concourse.bass as bass
import concourse.tile as tile
from concourse import bacc, mybir
from concourse.bass import ds, ts
from concourse.bass_utils import run_bass_kernel_spmd
from concourse.masks import make_identity

N_CORES = 8
B, C, D = 65536, 1000, 512
BS = B // N_CORES  # 8192 rows per core
CHUNKS = 8
CHUNK = BS // CHUNKS  # 1024 rows per chunk
SLOTS = CHUNK // 128  # 8
CS = C // N_CORES  # 125 gram rows per core

# ct = 2*radius(C-1)^2 - 1 from the reference, evaluated in f64, cast f32.
CT = float(np.float32(-0.0010010010010047532))

_F32 = mybir.dt.float32
_BF16 = mybir.dt.bfloat16
_I16 = mybir.dt.int16

_NC_CACHE = {}


def _build_body(ctx, tc, feat, centers, cslice, idx16, cbf, out):
    nc = tc.nc
    AF = mybir.ActivationFunctionType

    const = ctx.enter_context(tc.tile_pool(name="const", bufs=1))
    pang = ctx.enter_context(tc.tile_pool(name="ang", bufs=3))
    pnrm = ctx.enter_context(tc.tile_pool(name="nrm", bufs=3))
    pfeat = ctx.enter_context(tc.tile_pool(name="feat", bufs=2))
    pgath = ctx.enter_context(tc.tile_pool(name="gath", bufs=2))
    pscr = ctx.enter_context(tc.tile_pool(name="scr", bufs=2))
    ptp = ctx.enter_context(tc.tile_pool(name="tp", bufs=2, space="PSUM"))
    pgram = ctx.enter_context(tc.tile_pool(name="gram", bufs=2, space="PSUM"))
    pfin = ctx.enter_context(tc.tile_pool(name="fin", bufs=1, space="PSUM"))

    ident = const.tile([128, 128], _F32)
    make_identity(nc, ident[:])
    ones = const.tile([128, 1], _F32)
    nc.any.memset(ones[:], 1.0)
    # staging[:, 0:8]: per-chunk center-loss partials; [:, 8:10]: angular halves
    staging = const.tile([128, 16], _F32)
    nc.any.memset(staging[:], 0.0)
    ctbias = const.tile([128, 1], _F32)
    nc.any.memset(ctbias[:], -CT)

    cnT = const.tile([128, 4, C], _F32)  # normalized centers, transposed
    cnTs = const.tile([128, 4, CS], _F32)  # normalized slice, transposed

    idxt = const.tile([128, BS // 16], _I16)
    nc.sync.dma_start(idxt[:], idx16[:, :])

    def normalize_rows(t, p):
        """t: [p, D] rows in SBUF -> new tile with rows scaled to unit norm."""
        sq = pnrm.tile([128, D], _F32, tag="sqscr")
        nsq = pnrm.tile([128, 1], _F32, tag="nsq")
        nc.scalar.activation(sq[:p, :], t[:p, :], AF.Square, accum_out=nsq[:p, :])
        sd = pnrm.tile([128, 1], _F32, tag="sd")
        nc.scalar.activation(sd[:p, :], nsq[:p, :], AF.Sqrt)
        inv = pnrm.tile([128, 1], _F32, tag="inv")
        nc.vector.reciprocal(inv[:p, :], sd[:p, :])
        cn = pang.tile([128, D], _F32, tag="cnout")
        nc.scalar.activation(cn[:p, :], t[:p, :], AF.Copy, scale=inv[:p, :])
        return cn

    def transpose_into(dstT, cn, p, col0):
        """cn [p, D] -> dstT[:, ki, col0:col0+p] for each of the 4 d-blocks."""
        for ki in range(4):
            pt = ptp.tile([128, 128], _F32, tag="tp")
            nc.tensor.transpose(pt[:, :p], cn[:p, ts(ki, 128)], ident[:p, :p])
            nc.vector.tensor_copy(dstT[:, ki, col0 : col0 + p], pt[:, :p])

    # --- load centers once: write bf16 gather table + build normalized cnT ---
    col = 0
    while col < C:
        p = min(128, C - col)
        t = pang.tile([128, D], _F32, tag="cin")
        nc.sync.dma_start(t[:p, :], centers[ds(col, p), :])
        tb = pang.tile([128, D], _BF16, tag="cbf16")
        nc.vector.tensor_copy(tb[:p, :], t[:p, :])
        nc.sync.dma_start(cbf[ds(col, p), :], tb[:p, :])
        cn = normalize_rows(t, p)
        transpose_into(cnT, cn, p, col)
        col += p

    # --- center loss term ---
    for c in range(CHUNKS):
        ft = pfeat.tile([128, SLOTS, D], _F32, tag="ft")
        # partition p holds rows [c*CHUNK + p*SLOTS, ...+SLOTS): contiguous 16KB
        nc.sync.dma_start(
            ft[:], feat[ds(c * CHUNK, CHUNK), :].rearrange("(p s) d -> p s d", p=128)
        )
        gt = pgath.tile([128, SLOTS, D], _BF16, tag="gt")
        nc.gpsimd.dma_gather(
            gt[:],
            cbf[:, :],
            idxt[:, ds(c * (CHUNK // 16), CHUNK // 16)],
            CHUNK,
            CHUNK,
            D,
        )
        st = pscr.tile([128, SLOTS, D], _F32, tag="st")
        nc.vector.tensor_tensor(
            out=st[:], in0=ft[:], in1=gt[:], op=mybir.AluOpType.subtract
        )
        sqo = pgath.tile([128, SLOTS, D], _BF16, tag="sqo")
        nc.scalar.activation(sqo[:], st[:], AF.Square, accum_out=staging[:, c : c + 1])

    # --- angular term ---
    ts_ = pang.tile([128, D], _F32, tag="cin")
    nc.sync.dma_start(ts_[:CS, :], cslice[:, :])
    cns = normalize_rows(ts_, CS)
    transpose_into(cnTs, cns, CS, 0)

    NH = 2
    HW_ = C // NH  # 500
    for h in range(NH):
        pg = pgram.tile([CS, HW_], _F32, tag="gram")
        for ki in range(4):
            nc.tensor.matmul(
                pg[:],
                cnTs[:, ki, :],
                cnT[:, ki, ds(h * HW_, HW_)],
                start=(ki == 0),
                stop=(ki == 3),
            )
        gs = pnrm.tile([CS, HW_], _F32, tag="gscr")
        nc.scalar.activation(
            gs[:],
            pg[:],
            AF.Square,
            bias=ctbias[:CS, :],
            accum_out=staging[:CS, 8 + h : 9 + h],
        )

    # --- collapse partitions: out[0, j] = sum_p staging[p, j] ---
    pf = pfin.tile([1, 16], _F32, tag="fin")
    nc.tensor.matmul(pf[:], ones[:], staging[:], start=True, stop=True)
    osb = const.tile([1, 16], _F32)
    nc.vector.tensor_copy(osb[:], pf[:])
    nc.sync.dma_start(out[:, :], osb[:])


def build():
    if "nc" in _NC_CACHE:
        return _NC_CACHE["nc"]
    nc = bacc.Bacc(
        "TRN2",
        target_bir_lowering=False,
        debug=False,
        enable_asserts=False,
        num_devices=N_CORES,
    )
    feat = nc.dram_tensor("feat", [BS, D], _F32, kind="ExternalInput").ap()
    centers = nc.dram_tensor("centers", [C, D], _F32, kind="ExternalInput").ap()
    cslice = nc.dram_tensor("cslice", [CS, D], _F32, kind="ExternalInput").ap()
    idx16 = nc.dram_tensor("idx16", [128, BS // 16], _I16, kind="ExternalInput").ap()
    cbf = nc.dram_tensor("cbf", [C, D], _BF16).ap()  # internal bf16 gather table
    out = nc.dram_tensor("out", [1, 16], _F32, kind="ExternalOutput").ap()
    with tile.TileContext(nc) as tc, ExitStack() as ctx:
        _build_body(ctx, tc, feat, centers, cslice, idx16, cbf, out)
    nc.compile()
    _NC_CACHE["nc"] = nc
    return nc


def make_in_maps(y, feat, centers):
    feat = np.ascontiguousarray(feat, dtype=np.float32)
    centers = np.ascontiguousarray(centers, dtype=np.float32)
    y = np.asarray(y)
    in_maps = []
    for i in range(N_CORES):
        ys = y[i * BS : (i + 1) * BS].astype(np.int16)
        # gather position j in chunk c pairs with feat row c*CHUNK + (j%128)*SLOTS + j//128
        perm = np.arange(CHUNK)
        perm = (perm % 128) * SLOTS + perm // 128  # j -> row offset within chunk
        yp = ys.reshape(CHUNKS, CHUNK)[:, perm].reshape(-1)
        # [16, BS/16] stripes (position j at [j%16, j//16]), replicated into all
        # eight 16-partition groups (each SWDGE Q7 core reads its own stripe).
        idx = np.tile(yp.reshape(BS // 16, 16).T, (8, 1))
        in_maps.append(
            {
                "feat": np.ascontiguousarray(feat[i * BS : (i + 1) * BS]),
                "centers": centers,
                "cslice": np.ascontiguousarray(centers[i * CS : (i + 1) * CS]),
                "idx16": idx,
            }
        )
    return in_maps


def combine(outs):
    """outs: list of 8 [1,16] f32 arrays -> scalar loss (np.float32)."""
    cen = 0.0
    ang = 0.0
    for o in outs:
        o = np.asarray(o, dtype=np.float64)
        cen += o[0, 0:8].sum()
        ang += o[0, 8:10].sum()
    ang -= C * (1.0 - CT) ** 2  # remove the diagonal (sim_ii == 1) terms
    loss = 0.5 * cen / B + ang / (0.5 * C * (C - 1))
    return np.float32(loss)


def kernel(y, feat, centers):
    nc = build()
    in_maps = make_in_maps(y, feat, centers)
    res = run_bass_kernel_spmd(nc, in_maps, core_ids=list(range(N_CORES)))
    return combine([res.results[i]["out"] for i in range(N_CORES)])


# revision 8
# speedup vs baseline: 1.0252x; 1.0252x over previous
"""Trainium2 Bass kernel for AngelLoss (center loss + angular loss).

loss = 0.5*sum((feat - centers[y])^2)/B
     + sum_offdiag((c_i.c_j/(|c_i||c_j|) - ct)^2) / (0.5*C*(C-1))

Sharding (8 NeuronCores):
  - batch term: feat/y sharded along batch (8192 rows/core); each core
    dma_gathers its centers rows (bf16 table) and reduces to partials.
  - angular term: center rows sharded (125 rows/core); each core computes
    its 125x1000 slice of the normalized Gram matrix.
  - per-core [1,16] partial sums are combined on the host.
"""

from contextlib import ExitStack

import numpy as np

import concourse.bass as bass
import concourse.tile as tile
from concourse import bacc, mybir
from concourse.bass import ds, ts
from concourse.bass_utils import run_bass_kernel_spmd
from concourse.masks import make_identity

N_CORES = 8
B, C, D = 65536, 1000, 512
BS = B // N_CORES  # 8192 rows per core
CHUNKS = 8
CHUNK = BS // CHUNKS  # 1024 rows per chunk
SLOTS = CHUNK // 128  # 8
CS = C // N_CORES  # 125 gram rows per core

# ct = 2*radius(C-1)^2 - 1 from the reference, evaluated in f64, cast f32.
CT = float(np.float32(-0.0010010010010047532))

_F32 = mybir.dt.float32
_BF16 = mybir.dt.bfloat16
_I16 = mybir.dt.int16

_NC_CACHE = {}


def _build_body(ctx, tc, feat, centers, cslice, idx16, cbf, out):
    nc = tc.nc
    AF = mybir.ActivationFunctionType

    const = ctx.enter_context(tc.tile_pool(name="const", bufs=1))
    pcin = ctx.enter_context(tc.tile_pool(name="cin", bufs=9))
    pang = ctx.enter_context(tc.tile_pool(name="ang", bufs=3))
    pnrm = ctx.enter_context(tc.tile_pool(name="nrm", bufs=3))
    pfeat = ctx.enter_context(tc.tile_pool(name="feat", bufs=2))
    pgath = ctx.enter_context(tc.tile_pool(name="gath", bufs=2))
    pscr = ctx.enter_context(tc.tile_pool(name="scr", bufs=2))
    ptp = ctx.enter_context(tc.tile_pool(name="tp", bufs=2, space="PSUM"))
    pgram = ctx.enter_context(tc.tile_pool(name="gram", bufs=2, space="PSUM"))
    pfin = ctx.enter_context(tc.tile_pool(name="fin", bufs=1, space="PSUM"))

    ident = const.tile([128, 128], _F32)
    make_identity(nc, ident[:])
    ones = const.tile([128, 1], _F32)
    nc.any.memset(ones[:], 1.0)
    # staging[:, 0:8]: per-chunk center-loss partials; [:, 8:10]: angular halves
    staging = const.tile([128, 16], _F32)
    nc.any.memset(staging[:], 0.0)
    ctbias = const.tile([128, 1], _F32)
    nc.any.memset(ctbias[:], -CT)

    cnT = const.tile([128, 4, C], _F32)  # normalized centers, transposed
    cnTs = const.tile([128, 4, CS], _F32)  # normalized slice, transposed

    idxt = const.tile([128, BS // 16], _I16)
    nc.sync.dma_start(idxt[:], idx16[:, :])

    def normalize_rows(t, p):
        """t: [p, D] rows in SBUF -> new tile with rows scaled to unit norm."""
        sq = pnrm.tile([128, D], _F32, tag="sqscr")
        nsq = pnrm.tile([128, 1], _F32, tag="nsq")
        nc.scalar.activation(sq[:p, :], t[:p, :], AF.Square, accum_out=nsq[:p, :])
        sd = pnrm.tile([128, 1], _F32, tag="sd")
        nc.scalar.activation(sd[:p, :], nsq[:p, :], AF.Sqrt)
        inv = pnrm.tile([128, 1], _F32, tag="inv")
        nc.vector.reciprocal(inv[:p, :], sd[:p, :])
        cn = pang.tile([128, D], _F32, tag="cnout")
        nc.scalar.activation(cn[:p, :], t[:p, :], AF.Copy, scale=inv[:p, :])
        return cn

    def transpose_into(dstT, cn, p, col0):
        """cn [p, D] -> dstT[:, ki, col0:col0+p] for each of the 4 d-blocks."""
        for ki in range(4):
            pt = ptp.tile([128, 128], _F32, tag="tp")
            nc.tensor.transpose(pt[:, :p], cn[:p, ts(ki, 128)], ident[:p, :p])
            nc.vector.tensor_copy(dstT[:, ki, col0 : col0 + p], pt[:, :p])

    # --- phase 1: load centers, write bf16 gather table (nothing else on the
    # critical path, so the gathers can start ~immediately) ---
    ctiles = []
    col = 0
    while col < C:
        p = min(128, C - col)
        t = pcin.tile([128, D], _F32, tag="cin")
        nc.sync.dma_start(t[:p, :], centers[ds(col, p), :])
        tb = pang.tile([128, D], _BF16, tag="cbf16")
        nc.vector.tensor_copy(tb[:p, :], t[:p, :])
        nc.sync.dma_start(cbf[ds(col, p), :], tb[:p, :])
        ctiles.append((t, p, col))
        col += p
    tslice = pcin.tile([128, D], _F32, tag="cslice")
    nc.sync.dma_start(tslice[:CS, :], cslice[:, :])

    # --- phase 2: angular term from the resident center tiles (fills the
    # idle window while the first gathers + feat chunks are in flight) ---
    cns = normalize_rows(tslice, CS)
    transpose_into(cnTs, cns, CS, 0)
    for t, p, col in ctiles:
        cn = normalize_rows(t, p)
        transpose_into(cnT, cn, p, col)

    NH = 2
    HW_ = C // NH  # 500
    for h in range(NH):
        pg = pgram.tile([CS, HW_], _F32, tag="gram")
        for ki in range(4):
            nc.tensor.matmul(
                pg[:],
                cnTs[:, ki, :],
                cnT[:, ki, ds(h * HW_, HW_)],
                start=(ki == 0),
                stop=(ki == 3),
            )
        gs = pnrm.tile([CS, HW_], _F32, tag="gscr")
        nc.scalar.activation(
            gs[:],
            pg[:],
            AF.Square,
            bias=ctbias[:CS, :],
            accum_out=staging[:CS, 8 + h : 9 + h],
        )

    # --- phase 3: center-loss loop (gathers lead on the gpsimd stream) ---
    for c in range(CHUNKS):
        gt = pgath.tile([128, SLOTS, D], _BF16, tag="gt")
        nc.gpsimd.dma_gather(
            gt[:],
            cbf[:, :],
            idxt[:, ds(c * (CHUNK // 16), CHUNK // 16)],
            CHUNK,
            CHUNK,
            D,
        )
        ft = pfeat.tile([128, SLOTS, D], _F32, tag="ft")
        # partition p holds rows [c*CHUNK + p*SLOTS, ...+SLOTS): contiguous 16KB
        nc.sync.dma_start(
            ft[:], feat[ds(c * CHUNK, CHUNK), :].rearrange("(p s) d -> p s d", p=128)
        )
        st = pscr.tile([128, SLOTS, D], _F32, tag="st")
        nc.vector.tensor_tensor(
            out=st[:], in0=ft[:], in1=gt[:], op=mybir.AluOpType.subtract
        )
        sqo = pgath.tile([128, SLOTS, D], _BF16, tag="sqo")
        nc.scalar.activation(sqo[:], st[:], AF.Square, accum_out=staging[:, c : c + 1])

    # --- collapse partitions: out[0, j] = sum_p staging[p, j] ---
    pf = pfin.tile([1, 16], _F32, tag="fin")
    nc.tensor.matmul(pf[:], ones[:], staging[:], start=True, stop=True)
    osb = const.tile([1, 16], _F32)
    nc.vector.tensor_copy(osb[:], pf[:])
    nc.sync.dma_start(out[:, :], osb[:])


def build():
    if "nc" in _NC_CACHE:
        return _NC_CACHE["nc"]
    nc = bacc.Bacc(
        "TRN2",
        target_bir_lowering=False,
        debug=False,
        enable_asserts=False,
        num_devices=N_CORES,
    )
    feat = nc.dram_tensor("feat", [BS, D], _F32, kind="ExternalInput").ap()
    centers = nc.dram_tensor("centers", [C, D], _F32, kind="ExternalInput").ap()
    cslice = nc.dram_tensor("cslice", [CS, D], _F32, kind="ExternalInput").ap()
    idx16 = nc.dram_tensor("idx16", [128, BS // 16], _I16, kind="ExternalInput").ap()
    cbf = nc.dram_tensor("cbf", [C, D], _BF16).ap()  # internal bf16 gather table
    out = nc.dram_tensor("out", [1, 16], _F32, kind="ExternalOutput").ap()
    with tile.TileContext(nc) as tc, ExitStack() as ctx:
        _build_body(ctx, tc, feat, centers, cslice, idx16, cbf, out)
    nc.compile()
    _NC_CACHE["nc"] = nc
    return nc


def make_in_maps(y, feat, centers):
    feat = np.ascontiguousarray(feat, dtype=np.float32)
    centers = np.ascontiguousarray(centers, dtype=np.float32)
    y = np.asarray(y)
    in_maps = []
    for i in range(N_CORES):
        ys = y[i * BS : (i + 1) * BS].astype(np.int16)
        # gather position j in chunk c pairs with feat row c*CHUNK + (j%128)*SLOTS + j//128
        perm = np.arange(CHUNK)
        perm = (perm % 128) * SLOTS + perm // 128  # j -> row offset within chunk
        yp = ys.reshape(CHUNKS, CHUNK)[:, perm].reshape(-1)
        # [16, BS/16] stripes (position j at [j%16, j//16]), replicated into all
        # eight 16-partition groups (each SWDGE Q7 core reads its own stripe).
        idx = np.tile(yp.reshape(BS // 16, 16).T, (8, 1))
        in_maps.append(
            {
                "feat": np.ascontiguousarray(feat[i * BS : (i + 1) * BS]),
                "centers": centers,
                "cslice": np.ascontiguousarray(centers[i * CS : (i + 1) * CS]),
                "idx16": idx,
            }
        )
    return in_maps


def combine(outs):
    """outs: list of 8 [1,16] f32 arrays -> scalar loss (np.float32)."""
    cen = 0.0
    ang = 0.0
    for o in outs:
        o = np.asarray(o, dtype=np.float64)
        cen += o[0, 0:8].sum()
        ang += o[0, 8:10].sum()
    ang -= C * (1.0 - CT) ** 2  # remove the diagonal (sim_ii == 1) terms
    loss = 0.5 * cen / B + ang / (0.5 * C * (C - 1))
    return np.float32(loss)


def kernel(y, feat, centers):
    nc = build()
    in_maps = make_in_maps(y, feat, centers)
    res = run_bass_kernel_spmd(nc, in_maps, core_ids=list(range(N_CORES)))
    return combine([res.results[i]["out"] for i in range(N_CORES)])
